# revision 48
# baseline (speedup 1.0000x reference)
"""HardTripletMiningLoss on 8 TRN2 NeuronCores (Bass, raw-block SPMD).

Math: with emb = concat(anchor, positive, negative) [N,D], labels = ind[:,0],
pd(a,b) = ||e_a - e_b||^2, the loss is the mean over triplets (i,j,k) of
td = pd(i,j) - pd(j,k) + A restricted to
  same(i,j) & ~same(j,k) & td > 0 & i != 0.
Only (i,j) pairs with same labels (and i>=1) contribute — ~N^2/L of N^2 pairs.
Each such pair p=(i,j) needs, over k: sum/count of relu(td), where
  td(p,k) = 2*[g(j_p,k) - n_k/2 - (BIG/2)*same(j_p,k)] + (n_i - 2*u_p + A)
with g = emb gram, n = squared norms, u_p = <e_i, e_j>.

Per core: pairs become rows of [128, N] PSUM tiles V' via two accumulating
fp8(e4m3) matmuls: emb^T gathered by j against emb^T, plus a [1+L, .] aux
matmul whose lhs rows are (8.0 residual row, 8.0 one-hot(label_j)) and rhs
rows hold (-n_k/16 rounded to fp8, minus MASK/8 where labels match), with
row 0 correcting the fp8 rounding of -n_k/16.  fp8 rounding of emb gives
~3e-4 relative error on the final mean (tolerance is 2e-2; verified in a
numpy simulation that matches the hardware bit-for-bit).
Host precomputes per-pair thresholds t_p = -(n_i - 2u_p + A)/2:
  sum_k relu(td)/2 = relu-row-sum of (V' - t_p)  -> scalar engine ACT
  count_k          = row-sum of (V' > t_p)       -> vector engine DVE
running in parallel on the two engines.

Launch-overhead hygiene: three input DMAs issued in parallel from the sync
(emb) and scalar (aux, thresholds) queues; the tensor engine warms the HAM
clock gate with junk matmuls while they are in flight.  The relu-sum of the
last tile runs on DVE instead of ACT so the two reduction chains finish
together (ACT pays a ~1.3us table load before its first activation).  Host
sums the 8 cores' [128, 2T] partials and forms the mean.

Hazard notes from bringup: (1) the ACT function table may only be loaded by
the walrus-inserted load directly before the first activation — any attempt
to preload it earlier (a dummy activation in any form, or an explicit
InstLoadActFuncSet anywhere, even after all DMAs complete) reliably wedges
the device; (2) a tensor_scalar accumulates with op1, so relu-sum needs
scalar_tensor_tensor with a zeros tensor (op1=max would max-reduce the
accumulator).
"""

import numpy as np
from contextlib import ExitStack

import ml_dtypes

import concourse.bass as bass
import concourse.mybir as mybir
from concourse.bass_utils import run_bass_kernel_spmd

F32 = mybir.dt.float32
BF16 = mybir.dt.bfloat16
F8 = mybir.dt.float8e4            # ml_dtypes.float8_e4m3, max |x| = 240
AF = mybir.ActivationFunctionType
OP = mybir.AluOpType
NP_BF16 = ml_dtypes.bfloat16
NP_F8 = ml_dtypes.float8_e4m3

N_CORES = 8
A_MARGIN = 0.2
OH = 8.0           # one-hot scale: aux lhs entries are 8.0, rhs holds value/8
MASK = 1536.0      # same-label mask depth on V' (needs >:~700; col = -n/16-192)
PAD_THRESH = 0.5e9  # threshold for padding pair rows -> relu 0, count 0
MAX_TILES = 4      # per-core pair tiles per launch (PSUM bank budget)
NUM_WARM = 31      # junk matmuls to warm the PE clock gate during DMA wait

_programs: dict = {}  # (T, N, L) -> bass.Bass
LAST_RESULTS: list = []  # BassKernelResults of the launches in the last kernel() call


def _build_program(T: int, N: int, L: int) -> "bass.Bass":
    """One SPMD program: every core runs this with its own pair shard."""
    P = T * 128
    R = 1 + L        # aux contraction rows: residual row + one-hot label rows
    nc = bass.Bass()

    d_emb = nc.declare_dram_parameter("emb", [128, N + P], F8, isOutput=False)
    d_aux = nc.declare_dram_parameter("aux", [128, P + N], F8, isOutput=False)
    d_scal = nc.declare_dram_parameter("scal", [128, 2 * T], F32, isOutput=False)
    d_out = nc.declare_dram_parameter("out", [128, 2 * T], F32, isOutput=True)

    with ExitStack() as ctx:
        sb = lambda name, shape, dt: ctx.enter_context(nc.sbuf_tensor(name, shape, dt))
        ps = lambda name, shape: ctx.enter_context(nc.psum_tensor(name, shape, F32))

        emb_sb = sb("emb_sb", [128, N + P], F8)
        # rows 0:R live, rest zero pad ([65, x] DMA issue is ~2x slower than [128, x])
        aux_sb = sb("aux_sb", [128, P + N], F8)
        scal_sb = sb("scal_sb", [128, 2 * T], F32)  # [:, :T] -thresh, [:, T:] +thresh
        out_sb = sb("out_sb", [128, 2 * T], F32)    # [:, :T] relu sums, [:, T:] counts
        act_junk = sb("act_junk", [128, N], BF16)
        dve_junk = sb("dve_junk", [128, N], BF16)
        zeros_nt = sb("zeros_nt", [128, N], F32)
        stt_junk = sb("stt_junk", [128, N], F32)
        warm_w = sb("warm_w", [128, 128], BF16)     # uninitialized junk, warmup only

        psum_warm = ps("psum_warm", [128, 128])
        psumV = [ps(f"psumV{t}", [128, N]) for t in range(T)]

        n_act = max(T - 1, 0)  # tiles whose relu-sum runs on ACT; rest on DVE

        with (
            nc.semaphore("dma_in") as dma_in,
            nc.semaphore("dma_sc") as dma_sc,
            nc.semaphore("dma_out") as dma_out,
            nc.semaphore("mm") as mm,
            nc.semaphore("act_s") as act_s,
            nc.semaphore("dve_s") as dve_s,
            nc.Block() as block,
        ):

            @block.sync
            def _(sync):
                sync.dma_start(emb_sb[:], d_emb[:]).then_inc(dma_in, 16)
                sync.dma_start(scal_sb[:], d_scal[:]).then_inc(dma_in, 16)
                if n_act:
                    sync.wait_ge(act_s, n_act)
                sync.wait_ge(dve_s, 2 * T - n_act)
                # fire-and-forget: the block-end DRAIN already waits for this
                # engine's DMA queue to empty, so an explicit semaphore wait
                # only adds ~0.5us of completion-notification latency
                sync.dma_start(d_out[:], out_sb[:]).then_inc(dma_out, 16)

            @block.scalar
            def _(scalar):
                scalar.dma_start(aux_sb[:], d_aux[:]).then_inc(dma_sc, 16)
                # pre-wake on the DMA semaphores: the engine pays its ~0.5us
                # wake-from-idle latency here, off the critical path, so the
                # mm wait below is short/flow-through and the ACT table load
                # starts right at mm1 instead of 0.5us later
                scalar.wait_ge(dma_in, 32)
                scalar.wait_ge(dma_sc, 16)
                for t in range(n_act):
                    scalar.wait_ge(mm, t + 1)
                    # relu(V' - thresh) row-summed into out_sb[:, t]
                    nc.scalar.activation(
                        act_junk[:], psumV[t][:], AF.Relu,
                        bias=scal_sb[:, t:t + 1],
                        accum_out=out_sb[:, t:t + 1],
                    ).then_inc(act_s, 1)

            @block.tensor
            def _(tensor):
                for _ in range(NUM_WARM):
                    nc.tensor.matmul(
                        psum_warm[:], warm_w[:], warm_w[:], start=True, stop=True
                    )
                tensor.wait_ge(dma_in, 32)
                tensor.wait_ge(dma_sc, 16)
                for t in range(T):
                    nc.tensor.matmul(
                        psumV[t][:], emb_sb[:, N + t * 128:N + (t + 1) * 128],
                        emb_sb[:, 0:N], start=True, stop=False,
                    )
                    nc.tensor.matmul(
                        psumV[t][:], aux_sb[0:R, t * 128:(t + 1) * 128],
                        aux_sb[0:R, P:P + N], start=False, stop=True,
                    ).then_inc(mm, 1)

            @block.vector
            def _(vector):
                nc.vector.memset(zeros_nt[:], 0.0)
                vector.wait_ge(dma_in, 32)   # pre-wake, same as scalar above
                vector.wait_ge(dma_sc, 16)
                for t in range(T):
                    vector.wait_ge(mm, t + 1)
                    # count of V' > thresh row-summed into out_sb[:, T + t]
                    nc.vector.tensor_scalar(
                        dve_junk[:], psumV[t][:], scal_sb[:, T + t:T + t + 1], None,
                        OP.is_gt, OP.add, accum_out=out_sb[:, T + t:T + t + 1],
                    ).then_inc(dve_s, 1)
                for t in range(n_act, T):
                    # relu(V' - thresh) row-summed into out_sb[:, t] (sum accum
                    # needs scalar_tensor_tensor; tensor_scalar accums with op1)
                    nc.vector.scalar_tensor_tensor(
                        stt_junk[:], psumV[t][:], scal_sb[:, T + t:T + t + 1],
                        zeros_nt[:], OP.subtract, OP.max,
                        accum_out=out_sb[:, t:t + 1],
                    ).then_inc(dve_s, 1)

    return nc


def _get_program(T: int, N: int, L: int) -> "bass.Bass":
    key = (T, N, L)
    if key not in _programs:
        _programs[key] = _build_program(T, N, L)
    return _programs[key]


def _run_batch(emb_bf, labels, shared, ii, jj, thresh, T):
    """Run one SPMD launch over <=8*T*128 pairs; returns (sum, count) f64."""
    N = emb_bf.shape[1]
    L, auxr = shared
    R = 1 + L
    P = T * 128
    per = (len(ii) + N_CORES - 1) // N_CORES

    in_maps = []
    for c in range(N_CORES):
        sj = jj[c * per:(c + 1) * per]
        m = len(sj)
        emb_blob = np.zeros((128, N + P), NP_F8)
        emb_blob[:, :N] = emb_bf
        aux_blob = np.zeros((128, P + N), NP_F8)
        aux_blob[:R, P:] = auxr
        scal = np.empty((2 * T, 128), np.float32)  # [2T,128] then transpose
        scal[:T] = -PAD_THRESH
        scal[T:] = PAD_THRESH
        if m:
            emb_blob[:, N:N + m] = emb_bf[:, sj]
            aux_blob[0, :m] = OH
            aux_blob[1 + labels[sj], np.arange(m)] = OH
            th = thresh[c * per:c * per + m]
            flat = scal.reshape(2, T * 128)
            flat[0, :m] = -th
            flat[1, :m] = th
        scal_dev = np.ascontiguousarray(
            scal.reshape(2, T, 128).transpose(2, 0, 1).reshape(128, 2 * T))
        in_maps.append({"emb": emb_blob, "aux": aux_blob, "scal": scal_dev})

    nc = _get_program(T, N, L)
    res = run_bass_kernel_spmd(nc, in_maps, list(range(N_CORES)))
    LAST_RESULTS.append(res)
    s = 0.0
    cnt = 0.0
    for c in range(N_CORES):
        out = res.results[c]["out"].astype(np.float64)
        s += 2.0 * float(out[:, :T].sum())
        cnt += float(out[:, T:].sum())
    return s, cnt


def kernel(anchor, positive, negative, ind):
    LAST_RESULTS.clear()
    anchor = np.asarray(anchor, dtype=np.float32)
    positive = np.asarray(positive, dtype=np.float32)
    negative = np.asarray(negative, dtype=np.float32)
    labels = np.asarray(ind).reshape(-1).astype(np.int64)

    emb = np.ascontiguousarray(np.concatenate([anchor, positive, negative], axis=0))
    N, D = emb.shape
    assert D == 128, f"kernel assumes D=128, got {D}"
    assert N == labels.shape[0]

    L = int(labels.max()) + 1 if labels.size else 1
    assert L <= 127, f"label ids must fit one-hot partitions, got {L}"

    # same-label (i, j) pairs, excluding the i=0 plane (keep[0] = False)
    same = labels[:, None] == labels[None, :]
    ii, jj = np.nonzero(same)
    sel = ii >= 1
    ii, jj = ii[sel].astype(np.int64), jj[sel].astype(np.int64)

    if len(ii) == 0:
        return np.float32(0.0)

    n = (emb * emb).sum(axis=1, dtype=np.float32)          # squared norms [N]
    u = (emb[ii] * emb[jj]).sum(axis=1, dtype=np.float32)  # <e_i, e_j> per pair
    # thresh_p = -(n_i - 2 u_p + A)/2; hard iff V' > thresh_p
    thresh = (-0.5 * (n[ii] - 2.0 * u + A_MARGIN)).astype(np.float32)

    emb_bf = np.ascontiguousarray(emb.T).astype(NP_F8)     # [D, N]

    # aux rhs [1+L, N], all scaled by 1/OH (lhs one-hot entries are OH):
    # rows 1+l = fp8(-n/16), minus MASK/OH where labels==l; row 0 = residual
    # correcting the fp8 rounding of the -n/16 term.
    mk = (-n / (2.0 * OH)).astype(NP_F8)
    rk = ((-n / (2.0 * OH)) - mk.astype(np.float32)).astype(NP_F8)
    auxr = np.zeros((1 + L, N), NP_F8)
    auxr[0] = rk
    auxr[1:] = mk[None, :]
    lab_cols = np.arange(N)
    auxr[1 + labels, lab_cols] = ((-n[lab_cols] / (2.0 * OH)) - MASK / OH
                                  ).astype(NP_F8)
    shared = (L, auxr)

    batch_cap = N_CORES * MAX_TILES * 128
    s_tot, c_tot = 0.0, 0.0
    for b0 in range(0, len(ii), batch_cap):
        bi, bj = ii[b0:b0 + batch_cap], jj[b0:b0 + batch_cap]
        bt = thresh[b0:b0 + batch_cap]
        per = (len(bi) + N_CORES - 1) // N_CORES
        T = max(1, (per + 127) // 128)
        s, c = _run_batch(emb_bf, labels, shared, bi, bj, bt, T)
        s_tot += s
        c_tot += c

    if c_tot > 0:
        return np.float32(s_tot / max(c_tot, 1.0))
    return np.float32(0.0)
